# revision 28
# baseline (speedup 1.0000x reference)
"""AttnBlock (GroupNorm -> QKV -> full attention -> proj + residual) on 8
Trainium2 NeuronCores, data-parallel over the batch dimension (b=8, one
sample per core).

Layouts per core (sample):
  x:  (c=512, w=2048) fp32, channel tiles of 128 partitions.
  h:  GroupNorm(x) in f32r (feeds all matmuls).
  u = A.T h with A = (wq.T wk)/sqrt(c) folded on host (k never computed).
  Attention is computed TRANSPOSED: sT[j, i] = scores[i, j] via
  matmul(lhsT=h_tile, rhs=u_chunk) -- no PE transposes needed anywhere.
  exp applied by ACT straight out of PSUM into bf16 expT tiles; softmax
  denominators D[i] = sum_j expT[j, i] come from an all-ones [128,128]
  matmul (rows of the PSUM result are all D, i.e. pre-broadcast),
  accumulated over j-tiles in a rotating PSUM bank per i-chunk.
  ho = vp.T @ expT (vp = (wp wv).T-projected v, folded on host) runs
  fused per i-chunk right after that chunk's scores; normalization by
  1/D is applied to the OUTPUT columns, fused into the residual drain
  (x + bp prefetched by ACT, drain is 2 DVE ops).
  GroupNorm rsqrt is Exp(-0.5*Ln(v+eps)) so every activation in the
  kernel lives in the single natural_log_exp_and_others table set (no
  mid-kernel activation-table reloads).
  Biases folded exactly (bk/bq cross-terms cancel in softmax or become a
  per-partition bias g on u; bv/bp fold into bp_eff).
"""

import functools

import numpy as np

B = 8
C = 512
W = 2048
G = 32
EPS = 1e-6
P = 128
CT = C // P          # 4 channel tiles
NW = W // 512        # 4 w-chunks of 512
IT = W // P          # 16 tiles of 128 positions

TRACE = False
DEBUG = False
LAST_EXEC_NS = None
LAST_TRACE_PATH = None


def _build_nc():
    import concourse.bass as bass
    import concourse.mybir as mybir
    import concourse.tile as tile
    from concourse import bacc

    f32 = mybir.dt.float32
    f32r = mybir.dt.float32r
    bf16 = mybir.dt.bfloat16
    f8 = mybir.dt.float8e4
    DR = mybir.MatmulPerfMode.DoubleRow
    Ident = mybir.ActivationFunctionType.Identity
    Exp = mybir.ActivationFunctionType.Exp
    Sqrt = mybir.ActivationFunctionType.Sqrt
    mult = mybir.AluOpType.mult
    add = mybir.AluOpType.add
    subtract = mybir.AluOpType.subtract

    nc = bacc.Bacc()

    x_d = nc.declare_dram_parameter("x", [C, W], f32, isOutput=False)
    # Host-folded weights, partition-major [P, CT*C]:
    # A = (wq.T @ wk) * c^-0.5  (scores = h.T A h), WPV = (wp @ wv).T
    aT_d = nc.declare_dram_parameter("aT", [P, CT * C], f8, isOutput=False)
    wpvT_d = nc.declare_dram_parameter("wpvT", [P, CT * C], f8, isOutput=False)
    # One packed small-constant parameter (partition-major):
    # [0:512] per-tile group-avg selector S, [512:1024] selector-back ST,
    # then g, (pad), bp, gam, bet (CT cols each).
    aux_d = nc.declare_dram_parameter("aux", [P, 1044], f32, isOutput=False)
    out_d = nc.declare_dram_parameter("out", [C, W], f32, isOutput=True)

    with tile.TileContext(nc) as tc:
        with (
            tc.tile_pool(name="singles", bufs=1) as singles,
            tc.tile_pool(name="qk", bufs=1) as qkp,
            tc.tile_pool(name="vt", bufs=1) as vtp,
            tc.tile_pool(name="gn", bufs=2) as gnp,
        ):
            # wqkv holds the fp8 folded weights; it outlives psA because
            # the vp tail chains are interleaved into the attention phase.
            wqkv_cm = tc.tile_pool(name="wqkv", bufs=1)
            wqkv = wqkv_cm.__enter__()
            psA_cm = tc.tile_pool(name="psA", bufs=8, space="PSUM")
            psA = psA_cm.__enter__()
            a_sb = wqkv.tile([P, CT, C], f8, name="a_sb")
            wpv_sb = wqkv.tile([P, CT, C], f8, name="wpv_sb")
            # h and u live only in fp8, laid out [P, ct, W] so a 256-deep
            # contraction is 2 adjacent subtiles (DoubleRow: k = ct*128+p).
            # Weights are host-scaled (A*64, WPV*32) to sit in fp8 range;
            # the drains undo the scale.
            h8 = qkp.tile([P, CT, W], f8, name="h8")
            u8 = qkp.tile([P, CT, W], f8, name="u8")
            # x stays resident for the residual (no re-stream DMAs).
            x_sb = [qkp.tile([P, W], f32, name=f"x{t}") for t in range(CT)]

            # ---- DMAs: aux (small) then x tiles; weights follow x since
            # their first use is later than h0.
            ones_bf = singles.tile([P, P], bf16, name="ones_bf")
            nc.vector.memset(ones_bf, 1.0)
            eps_t = singles.tile([P, 1], f32, name="eps_t")
            nc.vector.memset(eps_t, EPS)
            aux_sb = singles.tile([P, 1044], f32, name="aux_sb")
            for hw in range(4):
                nc.sync.dma_start(
                    out=x_sb[0][:, hw * 512:(hw + 1) * 512],
                    in_=x_d[0 * P:1 * P, hw * 512:(hw + 1) * 512])
            nc.sync.dma_start(out=aux_sb, in_=aux_d[:, :])
            s_sb = aux_sb[:, 0:512].rearrange("p (t g) -> p t g", t=CT)
            st_sb = aux_sb[:, 512:1024].rearrange("p (t c) -> p t c", t=CT)
            g_sb = aux_sb[:, 1024:1028]
            bp_sb = aux_sb[:, 1032:1036]
            gam_sb = aux_sb[:, 1036:1040]
            bet_sb = aux_sb[:, 1040:1044]
            for hw in range(2):
                nc.sync.dma_start(
                    out=x_sb[1][:, hw * 1024:(hw + 1) * 1024],
                    in_=x_d[1 * P:2 * P, hw * 1024:(hw + 1) * 1024])
            nc.sync.dma_start(out=x_sb[2], in_=x_d[2 * P:3 * P, :])
            for hw in range(2):
                nc.sync.dma_start(
                    out=x_sb[3][:, hw * 1024:(hw + 1) * 1024],
                    in_=x_d[3 * P:4 * P, hw * 1024:(hw + 1) * 1024])
            nc.sync.dma_start(out=a_sb, in_=aT_d[:, :])
            nc.sync.dma_start(out=wpv_sb, in_=wpvT_d[:, :])

            # ===== GroupNorm: stats pass for all tiles first (keeps
            # DVE free of head-of-line blocking on the per-tile chains)
            st2a = singles.tile([P, CT, 2], f32, name="st2a")
            for t in range(CT):
                stats = gnp.tile([P, NW, 6], f32, tag="bnstats", name=f"bns{t}")
                for sg in range(NW):
                    nc.vector.bn_stats(out=stats[:, sg, :],
                                       in_=x_sb[t][:, sg * 512:(sg + 1) * 512])
                mv = gnp.tile([P, 2], f32, tag="mv", name=f"mv{t}")
                nc.vector.bn_aggr(out=mv, in_=stats)
                nc.vector.tensor_copy(out=st2a[:, t, 0:1], in_=mv[:, 0:1])
                nc.vector.tensor_tensor(out=st2a[:, t, 1:2], in0=mv[:, 0:1],
                                        in1=mv[:, 0:1], op=mult)
                nc.vector.tensor_add(out=st2a[:, t, 1:2], in0=st2a[:, t, 1:2],
                                     in1=mv[:, 1:2])

            def emit_gn_pair(pr):
                # One batched chain for tiles (2pr, 2pr+1): strided DVE ops
                # process both tiles at once (half the PE/ACT/DVE round
                # trips of per-tile chains).
                t0 = 2 * pr
                ps_g = psA.tile([P, 4], f32, tag="ps512", name=f"ps_g{pr}")
                for i in range(2):
                    nc.tensor.matmul(ps_g[:, 2 * i:2 * i + 2],
                                     lhsT=s_sb[:, t0 + i, :],
                                     rhs=st2a[:, t0 + i, :],
                                     start=True, stop=True)
                gsr = gnp.tile([P, 4], f32, tag="gsr", name=f"gsr{pr}")
                nc.vector.tensor_copy(out=gsr[:8, :], in_=ps_g[:8, :])
                gs2 = gnp.tile([P, 4], f32, tag="gs2", name=f"gs2_{pr}")
                nc.vector.memset(gs2, 0.0)
                nc.vector.tensor_copy(out=gs2[:8, 0::2], in_=gsr[:8, 0::2])
                var2 = gnp.tile([P, 2], f32, tag="var2", name=f"var2_{pr}")
                nc.vector.tensor_tensor(out=var2[:8, :], in0=gsr[:8, 0::2],
                                        in1=gsr[:8, 0::2], op=mult)
                nc.vector.tensor_tensor(out=var2[:8, :], in0=gsr[:8, 1::2],
                                        in1=var2[:8, :], op=subtract)
                # rsqrt(var+eps): Sqrt on ACT (its table set also covers
                # Identity, so the only set switches sit here in phase A,
                # not in the exp-paced attention sections) + a tiny DVE
                # reciprocal.
                nc.scalar.activation(out=var2[:8, :], in_=var2[:8, :],
                                     func=Sqrt, bias=eps_t[:8], scale=1.0)
                nc.vector.reciprocal(var2[:8, :], var2[:8, :])
                nc.vector.tensor_copy(out=gs2[:8, 1::2], in_=var2[:8, :])
                ps_bc = psA.tile([P, 4], f32, tag="ps512", name=f"psbc{pr}")
                for i in range(2):
                    nc.tensor.matmul(ps_bc[:, 2 * i:2 * i + 2],
                                     lhsT=st_sb[:, t0 + i, :],
                                     rhs=gs2[:, 2 * i:2 * i + 2],
                                     start=True, stop=True)
                bca = gnp.tile([P, 4], f32, tag="bca", name=f"bca{pr}")
                nc.vector.tensor_copy(out=bca, in_=ps_bc)
                alph = gnp.tile([P, 2], f32, tag=f"alph{pr}", name=f"alph{pr}")
                nc.vector.tensor_tensor(out=alph, in0=bca[:, 1::2],
                                        in1=gam_sb[:, t0:t0 + 2], op=mult)
                beta = gnp.tile([P, 2], f32, tag=f"beta{pr}", name=f"beta{pr}")
                nc.vector.tensor_tensor(out=beta, in0=bca[:, 0::2],
                                        in1=alph, op=mult)
                nc.vector.tensor_tensor(out=beta, in0=bet_sb[:, t0:t0 + 2],
                                        in1=beta, op=subtract)
                for i in range(2):
                    t = t0 + i
                    if t % 2 == 0:
                        nc.scalar.activation(out=h8[:, t, :], in_=x_sb[t],
                                             func=Ident,
                                             scale=alph[:, i:i + 1],
                                             bias=beta[:, i:i + 1])
                    else:
                        nc.vector.tensor_scalar(out=h8[:, t, :], in0=x_sb[t],
                                                scalar1=alph[:, i:i + 1],
                                                scalar2=beta[:, i:i + 1],
                                                op0=mult, op1=add)

            # ================= u = A.T h  and  vp = WPV.T h =========
            vp_sb = [vtp.tile([P, C], bf16, name=f"vp{j}") for j in range(IT)]

            def emit_phase(grp, pss, k):
                for ch in grp:
                    kind, a, b = ch
                    if kind == "u":
                        lhsT = a_sb[:, 2 * k:2 * k + 2, a * P:(a + 1) * P]
                        rhs = h8[:, 2 * k:2 * k + 2, b * 512:(b + 1) * 512]
                    else:
                        lhsT = h8[:, 2 * k:2 * k + 2, a * P:(a + 1) * P]
                        rhs = wpv_sb[:, 2 * k:2 * k + 2, :]
                    nc.tensor.matmul(pss[ch][:], lhsT=lhsT, rhs=rhs,
                                     start=(k == 0), stop=(k == 1),
                                     perf_mode=DR)

            def drain_chain(ch, pss):
                kind, a, b = ch
                if kind == "u":
                    nc.scalar.activation(
                        out=u8[:, a, b * 512:(b + 1) * 512],
                        in_=pss[ch], func=Ident,
                        bias=g_sb[:, a:a + 1], scale=1.0 / 64.0)
                else:
                    nc.vector.tensor_scalar_mul(out=vp_sb[a], in0=pss[ch],
                                                scalar1=1.0 / 32.0)

            # First 6 u-chains phase-woven with the GN tile chains.
            grp0 = ([("u", 0, jc) for jc in range(NW)]
                    + [("u", 1, 0), ("u", 1, 1)])
            pss0 = {}
            for ch in grp0:
                pss0[ch] = psA.tile([P, 512], f32, tag="ps512",
                                    name=f"psu0_{ch[1]}_{ch[2]}")
            for pr in range(2):
                emit_gn_pair(pr)
                emit_phase(grp0, pss0, pr)
            for ch in grp0:
                drain_chain(ch, pss0)

            # Remaining chains; the last 10 vp chains are NOT emitted here:
            # they become PE filler inside the first attention section.
            chains = ([("u", 1, 2), ("u", 1, 3)]
                      + [("u", ot, jc) for ot in range(2, CT) for jc in range(NW)]
                      + [("v", jt, 0) for jt in range(IT)])
            vp_tail = chains[16:26]
            bounds = [(0, 8), (8, 16)]
            for lo, hi in bounds:
                grp = chains[lo:hi]
                pss = {}
                for ch in grp:
                    pss[ch] = psA.tile(
                        [P, 512], f32, tag="ps512",
                        name=f"psqkv{ch[0]}{ch[1]}_{ch[2]}")
                for k in range(2):
                    emit_phase(grp, pss, k)
                for ch in grp:
                    drain_chain(ch, pss)

            psA_cm.__exit__(None, None, None)

            # ======== Attention: transposed scores, cross-chunk pipeline ====
            # Per i-chunk (ilc): the score section (DoubleRow fp8 sT + exp)
            # is ACT-throughput-bound, so the PREVIOUS chunk's D matmuls,
            # 1/D, ho chains and drains are interleaved into it as PE
            # filler.  The first section absorbs the leftover vp chains.
            expp_cm = tc.tile_pool(name="expp", bufs=1)
            expp = expp_cm.__enter__()
            expT = [expp.tile([P, W], bf16, name=f"e{jt}") for jt in range(IT)]
            rec_sb = expp.tile([P, W], f32, name="rec_sb")
            psd_cm = tc.tile_pool(name="psd", bufs=1, space="PSUM")
            psd = psd_cm.__enter__()
            psS_cm = tc.tile_pool(name="psS", bufs=2, space="PSUM")
            psS = psS_cm.__enter__()
            psO_cm = tc.tile_pool(name="psO", bufs=2, space="PSUM")
            psO = psO_cm.__enter__()
            outp_cm = tc.tile_pool(name="outp", bufs=2)
            outp = outp_cm.__enter__()

            def emit_d(d, ilc, jt):
                nc.tensor.matmul(d[:], lhsT=ones_bf,
                                 rhs=expT[jt][:, ilc * 512:(ilc + 1) * 512],
                                 start=(jt == 0), stop=(jt == IT - 1))

            def make_vp_chunks(tail):
                chunks = []
                for ch in tail:
                    def c(ch=ch):
                        _, a, b = ch
                        ps = psO.tile([P, 512], f32, tag="ps512",
                                      name=f"psvp{a}")
                        for k in range(2):
                            nc.tensor.matmul(
                                ps[:],
                                lhsT=h8[:, 2 * k:2 * k + 2, a * P:(a + 1) * P],
                                rhs=wpv_sb[:, 2 * k:2 * k + 2, :],
                                start=(k == 0), stop=(k == 1), perf_mode=DR)
                        nc.vector.tensor_scalar_mul(out=vp_sb[a], in0=ps,
                                                    scalar1=1.0 / 32.0)
                    chunks.append(c)
                return chunks

            def make_ho_chunks(d, ilc):
                chunks = []
                xbs = []
                pss = {}
                rslc = rec_sb[:, ilc * 512:(ilc + 1) * 512]

                def xb_prep():
                    for ot in range(CT):
                        xb = outp.tile([P, 512], f32, tag=f"xb{ot}",
                                       name=f"xb{ilc}_{ot}")
                        nc.vector.tensor_scalar(
                            out=xb,
                            in0=x_sb[ot][:, ilc * 512:(ilc + 1) * 512],
                            scalar1=bp_sb[:, ot:ot + 1], scalar2=None,
                            op0=add)
                        xbs.append(xb)
                chunks.append(xb_prep)
                for b in range(4):
                    def dchunk(b=b):
                        for j in range(4):
                            emit_d(d, ilc, 4 * b + j)
                    chunks.append(dchunk)

                def rec_():
                    # 1/D via the 2-NR-pass DVE approx (~18 correct bits,
                    # ~5x faster than reciprocal(), no ACT-table traffic).
                    nc.vector.reciprocal_approx_fast(out=rslc, in_=d[:])
                chunks.append(rec_)
                for ot in range(CT):
                    for seg in range(4):
                        def hoseg(ot=ot, seg=seg):
                            if seg == 0:
                                pss[ot] = psO.tile([P, 512], f32, tag="ps512",
                                                   name=f"ho{ilc}_{ot}")
                            for jt in range(4 * seg, 4 * seg + 4):
                                nc.tensor.matmul(
                                    pss[ot][:],
                                    lhsT=vp_sb[jt][:, ot * P:(ot + 1) * P],
                                    rhs=expT[jt][:, ilc * 512:(ilc + 1) * 512],
                                    start=(jt == 0), stop=(jt == IT - 1))
                        chunks.append(hoseg)

                    def drain(ot=ot):
                        t1 = outp.tile([P, 512], f32, tag="t1",
                                       name=f"t1_{ilc}_{ot}")
                        nc.vector.tensor_tensor(out=t1, in0=pss[ot],
                                                in1=rslc, op=mult)
                        osb = outp.tile([P, 512], f32, tag="osb",
                                        name=f"osb{ilc}_{ot}")
                        nc.vector.tensor_add(out=osb, in0=t1, in1=xbs[ot])
                        nc.sync.dma_start(
                            out=out_d[ot * P:(ot + 1) * P,
                                      ilc * 512:(ilc + 1) * 512],
                            in_=osb)
                    chunks.append(drain)
                return chunks

            def emit_sT_pair(p, filler):
                # Scores for i-chunk pair (2p, 2p+1): each 256-col fp8
                # LDWEIGHTS feeds TWO matmuls (the pair), halving the
                # weight-load wall; one [128,1024] exp per j-tile.
                done = 0
                for jt in range(IT):
                    ps = psS.tile([P, 1024], f32, tag="ps1024",
                                  name=f"st{jt}_p{p}")
                    for k in range(2):
                        for half in range(2):
                            ilc = 2 * p + half
                            nc.tensor.matmul(
                                ps[:, half * 512:(half + 1) * 512],
                                lhsT=h8[:, 2 * k:2 * k + 2,
                                        jt * P:(jt + 1) * P],
                                rhs=u8[:, 2 * k:2 * k + 2,
                                       ilc * 512:(ilc + 1) * 512],
                                start=(k == 0), stop=(k == 1), perf_mode=DR)
                    nc.scalar.activation(
                        out=expT[jt][:, p * 1024:(p + 1) * 1024],
                        in_=ps, func=Exp, bias=0.0, scale=1.0)
                    want = (len(filler) * (jt + 1)) // IT
                    while done < want:
                        filler[done]()
                        done += 1
                while done < len(filler):
                    filler[done]()
                    done += 1

            filler = make_vp_chunks(vp_tail)
            for p in range(2):
                dd = [psd.tile([P, 512], f32, tag=f"d{h}", name=f"d{2*p+h}")
                      for h in range(2)]
                emit_sT_pair(p, filler)
                filler = (make_ho_chunks(dd[0], 2 * p)
                          + make_ho_chunks(dd[1], 2 * p + 1))
            for c in filler:          # last pair's ho runs at the end
                c()

            outp_cm.__exit__(None, None, None)
            psO_cm.__exit__(None, None, None)
            psS_cm.__exit__(None, None, None)
            psd_cm.__exit__(None, None, None)
            expp_cm.__exit__(None, None, None)
            wqkv_cm.__exit__(None, None, None)

    nc.finalize()
    return nc


@functools.lru_cache(maxsize=1)
def _built():
    return _build_nc()


def _host_fold(x, gn_gamma, gn_beta, wq, bq, wk, bk, wv, bv, wp, bp):
    x = np.asarray(x, dtype=np.float32)
    scale = float(C) ** -0.5
    f = np.float32
    def pmajor(wT):
        # (C_in, C_out) -> [P, CT*C]: row p holds tiles t=0..CT-1 of wT
        return np.ascontiguousarray(
            wT.reshape(CT, P, C).transpose(1, 0, 2).reshape(P, CT * C))

    f64 = np.float64
    wq64 = np.asarray(wq, f64)
    wk64 = np.asarray(wk, f64)
    wv64 = np.asarray(wv, f64)
    wp64 = np.asarray(wp, f64)
    # scores = h.T A h + (wk.T bq~).h  (bk terms are per-row constants that
    # cancel in softmax); out_h = (wp wv h) attT
    import ml_dtypes
    f8np = ml_dtypes.float8_e4m3
    aT = pmajor((wq64.T @ wk64 * scale * 64.0).astype(f)).astype(f8np)
    wpvT = pmajor(((wp64 @ wv64).T * 32.0).astype(f)).astype(f8np)
    g_vec = (wk64.T @ (np.asarray(bq, f64) * scale)).astype(f)
    # v and out biases fold through the row-stochastic attention into bp
    bp_eff = (np.asarray(bp, f64) + wp64 @ np.asarray(bv, f64)).astype(f).reshape(C, 1)
    gam = np.asarray(gn_gamma, f).reshape(C, 1)
    bet = np.asarray(gn_beta, f).reshape(C, 1)

    gsz = C // G
    aux = np.zeros((P, 1044), dtype=f)
    for t in range(CT):
        for p in range(P):
            aux[p, t * P + p // gsz] = 1.0 / gsz          # S selector
            for cl in range(P):
                if p == cl // gsz:
                    aux[p, 512 + t * P + cl] = 1.0        # ST selector
    aux[:, 1024:1028] = g_vec.reshape(CT, P).T
    aux[:, 1032:1036] = bp_eff.reshape(CT, P).T
    aux[:, 1036:1040] = gam.reshape(CT, P).T
    aux[:, 1040:1044] = bet.reshape(CT, P).T

    shared = dict(aT=aT, wpvT=wpvT, aux=aux)
    return [dict(x=np.ascontiguousarray(x[i]), **shared) for i in range(B)]


def kernel(x, gn_gamma, gn_beta, wq, bq, wk, bk, wv, bv, wp, bp):
    global LAST_EXEC_NS, LAST_TRACE_PATH
    from concourse.bass_utils import run_bass_kernel_spmd

    in_maps = _host_fold(x, gn_gamma, gn_beta, wq, bq, wk, bk, wv, bv, wp, bp)
    nc = _built()
    last_err = None
    for attempt in range(3):
        try:
            res = run_bass_kernel_spmd(nc, in_maps, list(range(B)), trace=TRACE)
            out = np.stack([np.asarray(res.results[i]["out"], dtype=np.float32)
                            for i in range(B)], axis=0)
            break
        except Exception as e:  # transient NRT device errors: retry
            last_err = e
            if attempt == 2:
                raise
            import time
            time.sleep(2.0)
    if TRACE:
        LAST_EXEC_NS = res.exec_time_ns
        if res.instructions_and_trace is not None:
            LAST_TRACE_PATH = res.instructions_and_trace[1]
    return out


# revision 29
# speedup vs baseline: 1.1444x; 1.1444x over previous
"""AttnBlock (GroupNorm -> QKV -> full attention -> proj + residual) on 8
Trainium2 NeuronCores, data-parallel over the batch dimension (b=8, one
sample per core).

Layouts per core (sample):
  x:  (c=512, w=2048) fp32, channel tiles of 128 partitions.
  h:  GroupNorm(x) in f32r (feeds all matmuls).
  u = A.T h with A = (wq.T wk)/sqrt(c) folded on host (k never computed).
  Attention is computed TRANSPOSED: sT[j, i] = scores[i, j] via
  matmul(lhsT=h_tile, rhs=u_chunk) -- no PE transposes needed anywhere.
  exp applied by ACT straight out of PSUM into bf16 expT tiles; softmax
  denominators D[i] = sum_j expT[j, i] come from an all-ones [128,128]
  matmul (rows of the PSUM result are all D, i.e. pre-broadcast),
  accumulated over j-tiles in a rotating PSUM bank per i-chunk.
  ho = vp.T @ expT (vp = (wp wv).T-projected v, folded on host) runs
  fused per i-chunk right after that chunk's scores; normalization by
  1/D is applied to the OUTPUT columns, fused into the residual drain
  (x + bp prefetched by ACT, drain is 2 DVE ops).
  GroupNorm rsqrt is Exp(-0.5*Ln(v+eps)) so every activation in the
  kernel lives in the single natural_log_exp_and_others table set (no
  mid-kernel activation-table reloads).
  Biases folded exactly (bk/bq cross-terms cancel in softmax or become a
  per-partition bias g on u; bv/bp fold into bp_eff).
"""

import functools

import numpy as np

B = 8
C = 512
W = 2048
G = 32
EPS = 1e-6
P = 128
CT = C // P          # 4 channel tiles
NW = W // 512        # 4 w-chunks of 512
IT = W // P          # 16 tiles of 128 positions

TRACE = False
DEBUG = False
LAST_EXEC_NS = None
LAST_TRACE_PATH = None


def _build_nc():
    import concourse.bass as bass
    import concourse.mybir as mybir
    import concourse.tile as tile
    from concourse import bacc

    f32 = mybir.dt.float32
    f32r = mybir.dt.float32r
    bf16 = mybir.dt.bfloat16
    f8 = mybir.dt.float8e4
    DR = mybir.MatmulPerfMode.DoubleRow
    Ident = mybir.ActivationFunctionType.Identity
    Exp = mybir.ActivationFunctionType.Exp
    Sqrt = mybir.ActivationFunctionType.Sqrt
    mult = mybir.AluOpType.mult
    add = mybir.AluOpType.add
    subtract = mybir.AluOpType.subtract

    nc = bacc.Bacc()

    x_d = nc.declare_dram_parameter("x", [C, W], f32, isOutput=False)
    # Host-folded weights, partition-major [P, CT*C]:
    # A = (wq.T @ wk) * c^-0.5  (scores = h.T A h), WPV = (wp @ wv).T
    aT_d = nc.declare_dram_parameter("aT", [P, CT * C], f8, isOutput=False)
    wpvT_d = nc.declare_dram_parameter("wpvT", [P, CT * C], f8, isOutput=False)
    # One packed small-constant parameter (partition-major):
    # [0:512] per-tile group-avg selector S, [512:1024] selector-back ST,
    # then g, (pad), bp, gam, bet (CT cols each).
    aux_d = nc.declare_dram_parameter("aux", [P, 1044], f32, isOutput=False)
    out_d = nc.declare_dram_parameter("out", [C, W], f32, isOutput=True)

    with tile.TileContext(nc) as tc:
        with (
            tc.tile_pool(name="singles", bufs=1) as singles,
            tc.tile_pool(name="qk", bufs=1) as qkp,
            tc.tile_pool(name="vt", bufs=1) as vtp,
            tc.tile_pool(name="gn", bufs=2) as gnp,
        ):
            # wqkv holds the fp8 folded weights; it outlives psA because
            # the vp tail chains are interleaved into the attention phase.
            wqkv_cm = tc.tile_pool(name="wqkv", bufs=1)
            wqkv = wqkv_cm.__enter__()
            psA_cm = tc.tile_pool(name="psA", bufs=8, space="PSUM")
            psA = psA_cm.__enter__()
            a_sb = wqkv.tile([P, CT, C], f8, name="a_sb")
            wpv_sb = wqkv.tile([P, CT, C], f8, name="wpv_sb")
            # h and u live only in fp8, laid out [P, ct, W] so a 256-deep
            # contraction is 2 adjacent subtiles (DoubleRow: k = ct*128+p).
            # Weights are host-scaled (A*64, WPV*32) to sit in fp8 range;
            # the drains undo the scale.
            h8 = qkp.tile([P, CT, W], f8, name="h8")
            u8 = qkp.tile([P, CT, W], f8, name="u8")
            # x stays resident for the residual (no re-stream DMAs).
            x_sb = [qkp.tile([P, W], f32, name=f"x{t}") for t in range(CT)]

            # ---- DMAs: aux (small) then x tiles; weights follow x since
            # their first use is later than h0.
            ones_bf = singles.tile([P, P], bf16, name="ones_bf")
            nc.vector.memset(ones_bf, 1.0)
            eps_t = singles.tile([P, 1], f32, name="eps_t")
            nc.vector.memset(eps_t, EPS)
            aux_sb = singles.tile([P, 1044], f32, name="aux_sb")
            _dq = [nc.sync, nc.scalar]
            for hw in range(4):
                _dq[hw % 2].dma_start(
                    out=x_sb[0][:, hw * 512:(hw + 1) * 512],
                    in_=x_d[0 * P:1 * P, hw * 512:(hw + 1) * 512])
            nc.sync.dma_start(out=aux_sb, in_=aux_d[:, :])
            s_sb = aux_sb[:, 0:512].rearrange("p (t g) -> p t g", t=CT)
            st_sb = aux_sb[:, 512:1024].rearrange("p (t c) -> p t c", t=CT)
            g_sb = aux_sb[:, 1024:1028]
            bp_sb = aux_sb[:, 1032:1036]
            gam_sb = aux_sb[:, 1036:1040]
            bet_sb = aux_sb[:, 1040:1044]
            for hw in range(2):
                _dq[hw % 2].dma_start(
                    out=x_sb[1][:, hw * 1024:(hw + 1) * 1024],
                    in_=x_d[1 * P:2 * P, hw * 1024:(hw + 1) * 1024])
            for hw in range(2):
                _dq[hw % 2].dma_start(
                    out=x_sb[2][:, hw * 1024:(hw + 1) * 1024],
                    in_=x_d[2 * P:3 * P, hw * 1024:(hw + 1) * 1024])
            for hw in range(2):
                _dq[hw % 2].dma_start(
                    out=x_sb[3][:, hw * 1024:(hw + 1) * 1024],
                    in_=x_d[3 * P:4 * P, hw * 1024:(hw + 1) * 1024])
            nc.scalar.dma_start(out=a_sb, in_=aT_d[:, :])
            nc.sync.dma_start(out=wpv_sb, in_=wpvT_d[:, :])

            # ===== GroupNorm: stats pass for all tiles first (keeps
            # DVE free of head-of-line blocking on the per-tile chains)
            st2a = singles.tile([P, CT, 2], f32, name="st2a")
            for t in range(CT):
                stats = gnp.tile([P, NW, 6], f32, tag="bnstats", name=f"bns{t}")
                for sg in range(NW):
                    nc.vector.bn_stats(out=stats[:, sg, :],
                                       in_=x_sb[t][:, sg * 512:(sg + 1) * 512])
                mv = gnp.tile([P, 2], f32, tag="mv", name=f"mv{t}")
                nc.vector.bn_aggr(out=mv, in_=stats)
                nc.vector.tensor_copy(out=st2a[:, t, 0:1], in_=mv[:, 0:1])
                nc.vector.tensor_tensor(out=st2a[:, t, 1:2], in0=mv[:, 0:1],
                                        in1=mv[:, 0:1], op=mult)
                nc.vector.tensor_add(out=st2a[:, t, 1:2], in0=st2a[:, t, 1:2],
                                     in1=mv[:, 1:2])

            def emit_gn_pair(pr):
                # One batched chain for tiles (2pr, 2pr+1): strided DVE ops
                # process both tiles at once (half the PE/ACT/DVE round
                # trips of per-tile chains).
                t0 = 2 * pr
                ps_g = psA.tile([P, 4], f32, tag="ps512", name=f"ps_g{pr}")
                for i in range(2):
                    nc.tensor.matmul(ps_g[:, 2 * i:2 * i + 2],
                                     lhsT=s_sb[:, t0 + i, :],
                                     rhs=st2a[:, t0 + i, :],
                                     start=True, stop=True)
                gsr = gnp.tile([P, 4], f32, tag="gsr", name=f"gsr{pr}")
                nc.vector.tensor_copy(out=gsr[:8, :], in_=ps_g[:8, :])
                gs2 = gnp.tile([P, 4], f32, tag="gs2", name=f"gs2_{pr}")
                nc.vector.memset(gs2, 0.0)
                nc.vector.tensor_copy(out=gs2[:8, 0::2], in_=gsr[:8, 0::2])
                var2 = gnp.tile([P, 2], f32, tag="var2", name=f"var2_{pr}")
                nc.vector.tensor_tensor(out=var2[:8, :], in0=gsr[:8, 0::2],
                                        in1=gsr[:8, 0::2], op=mult)
                nc.vector.tensor_tensor(out=var2[:8, :], in0=gsr[:8, 1::2],
                                        in1=var2[:8, :], op=subtract)
                # rsqrt(var+eps): Sqrt on ACT (its table set also covers
                # Identity, so the only set switches sit here in phase A,
                # not in the exp-paced attention sections) + a tiny DVE
                # reciprocal.
                nc.scalar.activation(out=var2[:8, :], in_=var2[:8, :],
                                     func=Sqrt, bias=eps_t[:8], scale=1.0)
                nc.vector.reciprocal(var2[:8, :], var2[:8, :])
                nc.vector.tensor_copy(out=gs2[:8, 1::2], in_=var2[:8, :])
                ps_bc = psA.tile([P, 4], f32, tag="ps512", name=f"psbc{pr}")
                for i in range(2):
                    nc.tensor.matmul(ps_bc[:, 2 * i:2 * i + 2],
                                     lhsT=st_sb[:, t0 + i, :],
                                     rhs=gs2[:, 2 * i:2 * i + 2],
                                     start=True, stop=True)
                bca = gnp.tile([P, 4], f32, tag="bca", name=f"bca{pr}")
                nc.vector.tensor_copy(out=bca, in_=ps_bc)
                alph = gnp.tile([P, 2], f32, tag=f"alph{pr}", name=f"alph{pr}")
                nc.vector.tensor_tensor(out=alph, in0=bca[:, 1::2],
                                        in1=gam_sb[:, t0:t0 + 2], op=mult)
                beta = gnp.tile([P, 2], f32, tag=f"beta{pr}", name=f"beta{pr}")
                nc.vector.tensor_tensor(out=beta, in0=bca[:, 0::2],
                                        in1=alph, op=mult)
                nc.vector.tensor_tensor(out=beta, in0=bet_sb[:, t0:t0 + 2],
                                        in1=beta, op=subtract)
                for i in range(2):
                    t = t0 + i
                    if t % 2 == 0:
                        nc.scalar.activation(out=h8[:, t, :], in_=x_sb[t],
                                             func=Ident,
                                             scale=alph[:, i:i + 1],
                                             bias=beta[:, i:i + 1])
                    else:
                        nc.vector.tensor_scalar(out=h8[:, t, :], in0=x_sb[t],
                                                scalar1=alph[:, i:i + 1],
                                                scalar2=beta[:, i:i + 1],
                                                op0=mult, op1=add)

            # ================= u = A.T h  and  vp = WPV.T h =========
            vp_sb = [vtp.tile([P, C], bf16, name=f"vp{j}") for j in range(IT)]

            def emit_phase(grp, pss, k):
                for ch in grp:
                    kind, a, b = ch
                    if kind == "u":
                        lhsT = a_sb[:, 2 * k:2 * k + 2, a * P:(a + 1) * P]
                        rhs = h8[:, 2 * k:2 * k + 2, b * 512:(b + 1) * 512]
                    else:
                        lhsT = h8[:, 2 * k:2 * k + 2, a * P:(a + 1) * P]
                        rhs = wpv_sb[:, 2 * k:2 * k + 2, :]
                    nc.tensor.matmul(pss[ch][:], lhsT=lhsT, rhs=rhs,
                                     start=(k == 0), stop=(k == 1),
                                     perf_mode=DR)

            def drain_chain(ch, pss):
                kind, a, b = ch
                if kind == "u":
                    nc.scalar.activation(
                        out=u8[:, a, b * 512:(b + 1) * 512],
                        in_=pss[ch], func=Ident,
                        bias=g_sb[:, a:a + 1], scale=1.0 / 64.0)
                else:
                    nc.vector.tensor_scalar_mul(out=vp_sb[a], in0=pss[ch],
                                                scalar1=1.0 / 32.0)

            # First 6 u-chains phase-woven with the GN tile chains.
            grp0 = ([("u", 0, jc) for jc in range(NW)]
                    + [("u", 1, 0), ("u", 1, 1)])
            pss0 = {}
            for ch in grp0:
                pss0[ch] = psA.tile([P, 512], f32, tag="ps512",
                                    name=f"psu0_{ch[1]}_{ch[2]}")
            for pr in range(2):
                emit_gn_pair(pr)
                emit_phase(grp0, pss0, pr)
            for ch in grp0:
                drain_chain(ch, pss0)

            # Remaining chains; the last 10 vp chains are NOT emitted here:
            # they become PE filler inside the first attention section.
            chains = ([("u", 1, 2), ("u", 1, 3)]
                      + [("u", ot, jc) for ot in range(2, CT) for jc in range(NW)]
                      + [("v", jt, 0) for jt in range(IT)])
            vp_tail = chains[16:26]
            bounds = [(0, 8), (8, 16)]
            for lo, hi in bounds:
                grp = chains[lo:hi]
                pss = {}
                for ch in grp:
                    pss[ch] = psA.tile(
                        [P, 512], f32, tag="ps512",
                        name=f"psqkv{ch[0]}{ch[1]}_{ch[2]}")
                for k in range(2):
                    emit_phase(grp, pss, k)
                for ch in grp:
                    drain_chain(ch, pss)

            psA_cm.__exit__(None, None, None)

            # ======== Attention: transposed scores, cross-chunk pipeline ====
            # Per i-chunk (ilc): the score section (DoubleRow fp8 sT + exp)
            # is ACT-throughput-bound, so the PREVIOUS chunk's D matmuls,
            # 1/D, ho chains and drains are interleaved into it as PE
            # filler.  The first section absorbs the leftover vp chains.
            expp_cm = tc.tile_pool(name="expp", bufs=1)
            expp = expp_cm.__enter__()
            expT = [expp.tile([P, W], bf16, name=f"e{jt}") for jt in range(IT)]
            rec_sb = expp.tile([P, W], f32, name="rec_sb")
            psd_cm = tc.tile_pool(name="psd", bufs=1, space="PSUM")
            psd = psd_cm.__enter__()
            psS_cm = tc.tile_pool(name="psS", bufs=4, space="PSUM")
            psS = psS_cm.__enter__()
            psO_cm = tc.tile_pool(name="psO", bufs=3, space="PSUM")
            psO = psO_cm.__enter__()
            outp_cm = tc.tile_pool(name="outp", bufs=2)
            outp = outp_cm.__enter__()

            def emit_d(d, ilc, jt):
                nc.tensor.matmul(d[:], lhsT=ones_bf,
                                 rhs=expT[jt][:, ilc * 512:(ilc + 1) * 512],
                                 start=(jt == 0), stop=(jt == IT - 1))

            def make_vp_chunks(tail):
                chunks = []
                for ch in tail:
                    def c(ch=ch):
                        _, a, b = ch
                        ps = psO.tile([P, 512], f32, tag="ps512",
                                      name=f"psvp{a}")
                        for k in range(2):
                            nc.tensor.matmul(
                                ps[:],
                                lhsT=h8[:, 2 * k:2 * k + 2, a * P:(a + 1) * P],
                                rhs=wpv_sb[:, 2 * k:2 * k + 2, :],
                                start=(k == 0), stop=(k == 1), perf_mode=DR)
                        nc.vector.tensor_scalar_mul(out=vp_sb[a], in0=ps,
                                                    scalar1=1.0 / 32.0)
                    chunks.append(c)
                return chunks

            def make_ho_chunks(d, ilc):
                chunks = []
                xbs = []
                pss = {}
                stage = [None]
                rslc = rec_sb[:, ilc * 512:(ilc + 1) * 512]

                def xb_prep():
                    for ot in range(CT):
                        xb = outp.tile([P, 512], f32, tag=f"xb{ot}",
                                       name=f"xb{ilc}_{ot}")
                        nc.vector.tensor_scalar(
                            out=xb,
                            in0=x_sb[ot][:, ilc * 512:(ilc + 1) * 512],
                            scalar1=bp_sb[:, ot:ot + 1], scalar2=None,
                            op0=add)
                        xbs.append(xb)
                chunks.append(xb_prep)
                for b in range(4):
                    def dchunk(b=b):
                        for j in range(4):
                            emit_d(d, ilc, 4 * b + j)
                    chunks.append(dchunk)

                def rec_():
                    # 1/D via the 2-NR-pass DVE approx (~18 correct bits,
                    # ~5x faster than reciprocal(), no ACT-table traffic).
                    nc.vector.reciprocal_approx_fast(out=rslc, in_=d[:])
                chunks.append(rec_)
                for ot in range(CT):
                    for seg in range(4):
                        def hoseg(ot=ot, seg=seg):
                            if seg == 0:
                                pss[ot] = psO.tile([P, 512], f32, tag="ps512",
                                                   name=f"ho{ilc}_{ot}")
                            for jt in range(4 * seg, 4 * seg + 4):
                                nc.tensor.matmul(
                                    pss[ot][:],
                                    lhsT=vp_sb[jt][:, ot * P:(ot + 1) * P],
                                    rhs=expT[jt][:, ilc * 512:(ilc + 1) * 512],
                                    start=(jt == 0), stop=(jt == IT - 1))
                        chunks.append(hoseg)

                    def drain(ot=ot):
                        if ot == 0:
                            stage[0] = outp.tile([P, CT, 512], f32,
                                                 tag="stage",
                                                 name=f"stage{ilc}")
                        t1 = outp.tile([P, 512], f32, tag="t1",
                                       name=f"t1_{ilc}_{ot}")
                        nc.vector.tensor_tensor(out=t1, in0=pss[ot],
                                                in1=rslc, op=mult)
                        nc.vector.tensor_add(out=stage[0][:, ot, :],
                                             in0=t1, in1=xbs[ot])
                        if ot == CT - 1:
                            nc.sync.dma_start(
                                out=out_d[:, ilc * 512:(ilc + 1) * 512]
                                .rearrange("(t p) q -> p t q", t=CT),
                                in_=stage[0])
                    chunks.append(drain)
                return chunks

            def emit_sT_section(ilc, filler):
                done = 0
                for jt in range(IT):
                    ps = psS.tile([P, 512], f32, tag="ps512",
                                  name=f"st{jt}_{ilc}")
                    for k in range(2):
                        nc.tensor.matmul(
                            ps[:],
                            lhsT=h8[:, 2 * k:2 * k + 2, jt * P:(jt + 1) * P],
                            rhs=u8[:, 2 * k:2 * k + 2,
                                   ilc * 512:(ilc + 1) * 512],
                            start=(k == 0), stop=(k == 1), perf_mode=DR)
                    nc.scalar.activation(
                        out=expT[jt][:, ilc * 512:(ilc + 1) * 512],
                        in_=ps, func=Exp, bias=0.0, scale=1.0)
                    want = (len(filler) * (jt + 1)) // IT
                    while done < want:
                        filler[done]()
                        done += 1
                while done < len(filler):
                    filler[done]()
                    done += 1

            filler = make_vp_chunks(vp_tail)
            for ilc in range(NW):
                d = psd.tile([P, 512], f32, tag="d", name=f"d{ilc}")
                emit_sT_section(ilc, filler)
                filler = make_ho_chunks(d, ilc)
            for c in filler:          # last chunk's ho runs at the end
                c()

            outp_cm.__exit__(None, None, None)
            psO_cm.__exit__(None, None, None)
            psS_cm.__exit__(None, None, None)
            psd_cm.__exit__(None, None, None)
            expp_cm.__exit__(None, None, None)
            wqkv_cm.__exit__(None, None, None)

    nc.finalize()
    return nc


@functools.lru_cache(maxsize=1)
def _built():
    return _build_nc()


def _host_fold(x, gn_gamma, gn_beta, wq, bq, wk, bk, wv, bv, wp, bp):
    x = np.asarray(x, dtype=np.float32)
    scale = float(C) ** -0.5
    f = np.float32
    def pmajor(wT):
        # (C_in, C_out) -> [P, CT*C]: row p holds tiles t=0..CT-1 of wT
        return np.ascontiguousarray(
            wT.reshape(CT, P, C).transpose(1, 0, 2).reshape(P, CT * C))

    f64 = np.float64
    wq64 = np.asarray(wq, f64)
    wk64 = np.asarray(wk, f64)
    wv64 = np.asarray(wv, f64)
    wp64 = np.asarray(wp, f64)
    # scores = h.T A h + (wk.T bq~).h  (bk terms are per-row constants that
    # cancel in softmax); out_h = (wp wv h) attT
    import ml_dtypes
    f8np = ml_dtypes.float8_e4m3
    aT = pmajor((wq64.T @ wk64 * scale * 64.0).astype(f)).astype(f8np)
    wpvT = pmajor(((wp64 @ wv64).T * 32.0).astype(f)).astype(f8np)
    g_vec = (wk64.T @ (np.asarray(bq, f64) * scale)).astype(f)
    # v and out biases fold through the row-stochastic attention into bp
    bp_eff = (np.asarray(bp, f64) + wp64 @ np.asarray(bv, f64)).astype(f).reshape(C, 1)
    gam = np.asarray(gn_gamma, f).reshape(C, 1)
    bet = np.asarray(gn_beta, f).reshape(C, 1)

    gsz = C // G
    aux = np.zeros((P, 1044), dtype=f)
    for t in range(CT):
        for p in range(P):
            aux[p, t * P + p // gsz] = 1.0 / gsz          # S selector
            for cl in range(P):
                if p == cl // gsz:
                    aux[p, 512 + t * P + cl] = 1.0        # ST selector
    aux[:, 1024:1028] = g_vec.reshape(CT, P).T
    aux[:, 1032:1036] = bp_eff.reshape(CT, P).T
    aux[:, 1036:1040] = gam.reshape(CT, P).T
    aux[:, 1040:1044] = bet.reshape(CT, P).T

    shared = dict(aT=aT, wpvT=wpvT, aux=aux)
    return [dict(x=np.ascontiguousarray(x[i]), **shared) for i in range(B)]


def kernel(x, gn_gamma, gn_beta, wq, bq, wk, bk, wv, bv, wp, bp):
    global LAST_EXEC_NS, LAST_TRACE_PATH
    from concourse.bass_utils import run_bass_kernel_spmd

    in_maps = _host_fold(x, gn_gamma, gn_beta, wq, bq, wk, bk, wv, bv, wp, bp)
    nc = _built()
    last_err = None
    for attempt in range(3):
        try:
            res = run_bass_kernel_spmd(nc, in_maps, list(range(B)), trace=TRACE)
            out = np.stack([np.asarray(res.results[i]["out"], dtype=np.float32)
                            for i in range(B)], axis=0)
            break
        except Exception as e:  # transient NRT device errors: retry
            last_err = e
            if attempt == 2:
                raise
            import time
            time.sleep(2.0)
    if TRACE:
        LAST_EXEC_NS = res.exec_time_ns
        if res.instructions_and_trace is not None:
            LAST_TRACE_PATH = res.instructions_and_trace[1]
    return out
